# revision 1
# baseline (speedup 1.0000x reference)
"""DLRM (multi-table EmbeddingBag + MLPs) on 8 Trainium2 NeuronCores.

Strategy: data-parallel over batch (512 bags/core); embedding tables
replicated in each core's HBM as 104 window tensors (4 windows x 25000 rows
per table, so local row ids fit dma_gather's int16 index limit). The host
only reorders/pads index metadata; all table rows are fetched on-device via
dma_gather (4 SWDGE queues). Pooling uses a per-128-slot-group selection
matmul (gathered rows as stationary lhsT, on-chip one-hot bag matrix as
moving rhs) accumulating into PSUM - the same duplicate-safe reduction
pattern as tile_scatter_add. Bottom/top MLPs run fused in the same kernel,
feature-major, in fp32.
"""
import numpy as np

import concourse.bacc as bacc
import concourse.bass as bass
import concourse.mybir as mybir
import concourse.tile as tile
from concourse.bass_utils import run_bass_kernel_spmd

T = 26          # tables
R = 100000      # rows per table
E = 64          # embedding dim
B = 4096        # batch
L = 32          # lookups per bag
BOT = [256, 512, 256, 64]
TOP = [E * (1 + T), 512, 256, 1]   # 1728 -> 512 -> 256 -> 1
N_CORES = 8
B_CORE = B // N_CORES               # 512 bags per core
N_BT = B_CORE // 128                # 4 bag-tiles per core
W_ROWS = 25000                      # window rows (<= int16 range)
N_WIN = R // W_ROWS                 # 4 windows per table
P = 128
ZF = TOP[0] + 64                    # 1792: zero-padded feature dim
NZCH = ZF // P                      # 14 z chunks


def _pack_idx_block(idx_i16):
    """[n] int16 (n % 16 == 0) -> [128, n//16]: j -> (j%16, j//16), replicated x8."""
    n = idx_i16.size
    w = idx_i16.reshape(n // 16, 16).T
    return np.tile(w, (8, 1))


def _chunk_weights(wt):
    """W.T [din, dout] -> [128, (din/128)*dout] SBUF chunk layout."""
    din, dout = wt.shape
    nk = din // P
    return np.ascontiguousarray(wt.reshape(nk, P, dout).transpose(1, 0, 2).reshape(P, nk * dout))


def _chunk_bias(b):
    """[dout] -> [128, ceil(dout/128)]."""
    dout = b.size
    nch = -(-dout // P)
    buf = np.zeros(nch * P, np.float32)
    buf[:dout] = b
    return np.ascontiguousarray(buf.reshape(nch, P).T)


def _host_prep(x_indices):
    """Per-core segment packing. Returns caps (shared) + per-core idx/bagid arrays."""
    idx = np.asarray(x_indices).astype(np.int64)  # [T, B, L]
    per_core = []
    for c in range(N_CORES):
        core_segs = []
        idx_c = idx[:, c * B_CORE:(c + 1) * B_CORE, :]        # [T, 512, L]
        win = idx_c // W_ROWS
        for bt in range(N_BT):
            sub = idx_c[:, bt * 128:(bt + 1) * 128, :]        # [T, 128, L]
            wsub = win[:, bt * 128:(bt + 1) * 128, :]
            for t in range(T):
                for w in range(N_WIN):
                    bags, ls = np.nonzero(wsub[t] == w)
                    li = (sub[t][bags, ls] - w * W_ROWS).astype(np.int16)
                    core_segs.append((li, bags.astype(np.float32)))
        per_core.append(core_segs)
    n_segs = len(per_core[0])
    caps = []
    for s in range(n_segs):
        m = max(per_core[c][s][0].size for c in range(N_CORES))
        caps.append(max(128, -(-m // 128) * 128))
    assert max(caps) <= 8192
    tot16 = sum(cp // 16 for cp in caps)
    gtot = sum(cp // 128 for cp in caps)
    idx_mats, bag_mats = [], []
    for c in range(N_CORES):
        im = np.zeros((P, tot16), np.int16)
        bm = np.full((P, gtot), 255.0, np.float32)
        o16 = 0
        og = 0
        for s, cp in enumerate(caps):
            li, bags = per_core[c][s]
            buf = np.zeros(cp, np.int16)
            buf[:li.size] = li
            im[:, o16:o16 + cp // 16] = _pack_idx_block(buf)
            bb = np.full(cp, 255.0, np.float32)
            bb[:bags.size] = bags
            bm[:, og:og + cp // 128] = bb.reshape(cp // 128, P).T
            o16 += cp // 16
            og += cp // 128
        idx_mats.append(im)
        bag_mats.append(bm)
    return caps, idx_mats, bag_mats, tot16, gtot


def _build(caps, tot16, gtot):
    nc = bacc.Bacc("TRN2", target_bir_lowering=False, debug=False,
                   enable_asserts=False, num_devices=N_CORES, num_swdge_queues=4)
    dt = mybir.dt.float32
    AF = mybir.ActivationFunctionType

    win_d = [nc.dram_tensor(f"win{t}_{w}", [W_ROWS, E], dt, kind="ExternalInput").ap()
             for t in range(T) for w in range(N_WIN)]
    idx_d = nc.dram_tensor("idxs", [P, tot16], mybir.dt.int16, kind="ExternalInput").ap()
    bag_d = nc.dram_tensor("bagids", [P, gtot], dt, kind="ExternalInput").ap()
    xt_d = nc.dram_tensor("xt", [BOT[0], B_CORE], dt, kind="ExternalInput").ap()
    # weights pre-chunked on host: [128, nk*dout]; biases [128, nch]
    wdims = [(BOT[0], BOT[1]), (BOT[1], BOT[2]), (BOT[2], BOT[3]),
             (ZF, TOP[1]), (TOP[1], TOP[2]), (TOP[2], TOP[3])]
    w_d = [nc.dram_tensor(f"w{i}", [P, (din // P) * dout], dt, kind="ExternalInput").ap()
           for i, (din, dout) in enumerate(wdims)]
    b_d = [nc.dram_tensor(f"b{i}", [P, -(-dout // P)], dt, kind="ExternalInput").ap()
           for i, (_, dout) in enumerate(wdims)]
    out_d = nc.dram_tensor("y", [1, B_CORE], dt, kind="ExternalOutput").ap()

    with tile.TileContext(nc) as tc:
        with tc.tile_pool(name="const", bufs=1) as cpool, \
             tc.tile_pool(name="zp", bufs=1) as zp, \
             tc.tile_pool(name="stg", bufs=4) as stg, \
             tc.tile_pool(name="gp", bufs=6) as gp, \
             tc.tile_pool(name="sp", bufs=6) as sp, \
             tc.tile_pool(name="act", bufs=1) as actp, \
             tc.tile_pool(name="pps", bufs=2, space="PSUM") as pps, \
             tc.tile_pool(name="mps", bufs=2, space="PSUM") as mps:

            iota = cpool.tile([P, P], dt)
            nc.gpsimd.iota(iota[:], pattern=[[1, P]], base=0, channel_multiplier=0,
                           allow_small_or_imprecise_dtypes=True)

            bagid = cpool.tile([P, gtot], dt)
            nc.sync.dma_start(out=bagid[:], in_=bag_d[:])

            zt = zp.tile([P, NZCH * B_CORE], dt)
            nc.vector.memset(zt[:], 0.0)

            def load(name, ap_dram, shape):
                t_ = cpool.tile(shape, dt, tag=name)
                nc.sync.dma_start(out=t_[:], in_=ap_dram)
                return t_

            xt = [load(f"xt{k}", xt_d[k * P:(k + 1) * P, :], [P, B_CORE])
                  for k in range(BOT[0] // P)]
            wts = [load(f"w{i}", w_d[i][:, :], [P, (din // P) * dout])
                   for i, (din, dout) in enumerate(wdims)]
            bts = [load(f"b{i}", b_d[i][:, :], [P, -(-dout // P)])
                   for i, (_, dout) in enumerate(wdims)]

            def mlp_layer(src_aps, li, func, out_tag):
                din, dout = wdims[li]
                nk = din // P
                outs = []
                for m in range(-(-dout // P)):
                    mm = min(P, dout - m * P)
                    ps = mps.tile([P, B_CORE], dt, space="PSUM", tag="mlp")
                    for k in range(nk):
                        nc.tensor.matmul(
                            out=ps[:mm, :],
                            lhsT=wts[li][:, k * dout + m * P: k * dout + m * P + mm],
                            rhs=src_aps[k],
                            start=(k == 0), stop=(k == nk - 1))
                    o = actp.tile([P, B_CORE], dt, tag=f"{out_tag}{m}")
                    nc.scalar.activation(out=o[:mm, :], in_=ps[:mm, :], func=func,
                                         bias=bts[li][0:mm, m:m + 1])
                    outs.append(o)
                return outs

            # bottom MLP (feature-major h.T tiles [128, 512])
            h1 = mlp_layer([t_[:, :] for t_ in xt], 0, AF.Relu, "h1")
            h2 = mlp_layer([t_[:, :] for t_ in h1], 1, AF.Relu, "h2")
            h3 = mlp_layer([t_[:, :] for t_ in h2], 2, AF.Relu, "h3")
            nc.vector.tensor_copy(out=zt[0:64, 0:B_CORE], in_=h3[0][0:64, :])

            # embedding gather + selection-matmul pooling
            o16 = 0
            og = 0
            si = 0
            for bt in range(N_BT):
                for t in range(T):
                    fbase = 64 + 64 * t
                    ch = fbase // P
                    prow = fbase % P
                    pooled = pps.tile([P, P], dt, space="PSUM", tag="pooled")
                    n_seg_mm = sum(caps[si + w] // 128 for w in range(N_WIN))
                    mm_i = 0
                    for w in range(N_WIN):
                        cp = caps[si + w]
                        c16 = cp // 16
                        gn = cp // 128
                        stage = stg.tile([P, c16], mybir.dt.int16, tag="stage")
                        nc.sync.dma_start(out=stage[:], in_=idx_d[:, o16:o16 + c16])
                        idx_t = stg.tile([P, c16], mybir.dt.int16, tag="idxt")
                        nc.vector.tensor_copy(out=idx_t[:], in_=stage[:])
                        dst = gp.tile([P, gn, E], dt, tag="dst")
                        nc.gpsimd.dma_gather(
                            out_ap=dst[:], in_ap=win_d[t * N_WIN + w][:],
                            idxs_ap=idx_t[:], num_idxs=cp, num_idxs_reg=cp,
                            elem_size=E, single_packet=False,
                            queue_num=(si + w) % 4)
                        sel = sp.tile([P, gn, P], dt, tag="sel")
                        nc.vector.tensor_tensor(
                            out=sel[:],
                            in0=bagid[:, og:og + gn, None].to_broadcast([P, gn, P]),
                            in1=iota[:, None, :].to_broadcast([P, gn, P]),
                            op=mybir.AluOpType.is_equal)
                        for g in range(gn):
                            nc.tensor.matmul(
                                out=pooled[prow:prow + 64, :],
                                lhsT=dst[:, g, :], rhs=sel[:, g, :],
                                start=(mm_i == 0), stop=(mm_i == n_seg_mm - 1))
                            mm_i += 1
                        o16 += c16
                        og += gn
                    si += N_WIN
                    nc.vector.tensor_copy(
                        out=zt[prow:prow + 64,
                               ch * B_CORE + bt * P: ch * B_CORE + bt * P + P],
                        in_=pooled[prow:prow + 64, :])

            # top MLP
            zsrc = [zt[:, c * B_CORE:(c + 1) * B_CORE] for c in range(NZCH)]
            y1 = mlp_layer(zsrc, 3, AF.Relu, "y1")
            y2 = mlp_layer([t_[:, :] for t_ in y1], 4, AF.Relu, "y2")
            ps = mps.tile([P, B_CORE], dt, space="PSUM", tag="mlp")
            nk = TOP[2] // P
            for k in range(nk):
                nc.tensor.matmul(out=ps[:1, :],
                                 lhsT=wts[5][:, k * TOP[3]: k * TOP[3] + 1],
                                 rhs=y2[k][:, :],
                                 start=(k == 0), stop=(k == nk - 1))
            yo = actp.tile([1, B_CORE], dt, tag="yo")
            nc.scalar.activation(out=yo[:], in_=ps[:1, :], func=AF.Sigmoid,
                                 bias=bts[5][0:1, 0:1])
            nc.sync.dma_start(out=out_d[:], in_=yo[:])

    nc.compile()
    return nc


def kernel(**inputs):
    x_dense = np.asarray(inputs["x_dense"], np.float32)
    x_indices = np.asarray(inputs["x_indices"])
    emb = np.ascontiguousarray(np.asarray(inputs["emb_tables"], np.float32))

    caps, idx_mats, bag_mats, tot16, gtot = _host_prep(x_indices)
    nc = _build(caps, tot16, gtot)

    common = {}
    for t in range(T):
        for w in range(N_WIN):
            common[f"win{t}_{w}"] = np.ascontiguousarray(
                emb[t, w * W_ROWS:(w + 1) * W_ROWS, :])
    w0 = np.asarray(inputs["top_w0"], np.float32)          # [512, 1728]
    w0p = np.zeros((TOP[1], ZF), np.float32)
    w0p[:, :TOP[0]] = w0
    wlist = [np.asarray(inputs["bot_w0"], np.float32).T,
             np.asarray(inputs["bot_w1"], np.float32).T,
             np.asarray(inputs["bot_w2"], np.float32).T,
             w0p.T,
             np.asarray(inputs["top_w1"], np.float32).T,
             np.asarray(inputs["top_w2"], np.float32).T]
    blist = [np.asarray(inputs["bot_b0"], np.float32),
             np.asarray(inputs["bot_b1"], np.float32),
             np.asarray(inputs["bot_b2"], np.float32),
             np.asarray(inputs["top_b0"], np.float32),
             np.asarray(inputs["top_b1"], np.float32),
             np.asarray(inputs["top_b2"], np.float32)]
    for i in range(6):
        common[f"w{i}"] = _chunk_weights(wlist[i])
        common[f"b{i}"] = _chunk_bias(blist[i])

    in_maps = []
    for c in range(N_CORES):
        m = dict(common)
        m["idxs"] = idx_mats[c]
        m["bagids"] = bag_mats[c]
        m["xt"] = np.ascontiguousarray(x_dense[c * B_CORE:(c + 1) * B_CORE, :].T)
        in_maps.append(m)

    res = run_bass_kernel_spmd(nc, in_maps, core_ids=list(range(N_CORES)))
    y = np.empty((B, 1), np.float32)
    for c in range(N_CORES):
        y[c * B_CORE:(c + 1) * B_CORE, 0] = res.results[c]["y"][0]
    return y



# revision 13
# speedup vs baseline: 54.3408x; 54.3408x over previous
"""DLRM (multi-table EmbeddingBag + MLPs) on 8 Trainium2 NeuronCores.

Data-parallel over batch (512 bags/core); embedding tables replicated in
each core's HBM as 104 window tensors ([25000, 128] bf16, cols 64:128
zero-padded so each row is one 256B gather element; 25000 rows keeps local
ids within dma_gather's int16 range). One dma_gather per (table, window)
fetches the window's rows for all 4 bag-tiles at once (segments per
bag-tile padded to 128-multiples). Pooling uses per-128-slot-group
selection matmuls in bf16 (gathered rows as stationary lhsT, on-chip
one-hot bag matrix as moving rhs) accumulating into per-bag-tile PSUM
tiles across the 4 windows. Bottom/top MLPs run fused in fp32,
feature-major.
"""
import numpy as np
import ml_dtypes

import concourse.bacc as bacc
import concourse.bass as bass
import concourse.mybir as mybir
import concourse.tile as tile
from concourse.bass_utils import run_bass_kernel_spmd

T = 26          # tables
R = 100000      # rows per table
E = 64          # embedding dim
B = 4096        # batch
L = 32          # lookups per bag
BOT = [256, 512, 256, 64]
TOP = [E * (1 + T), 512, 256, 1]   # 1728 -> 512 -> 256 -> 1
N_CORES = 8
B_CORE = B // N_CORES               # 512 bags per core
N_BT = B_CORE // 128                # 4 bag-tiles per core
W_ROWS = 25000                      # window rows (<= int16 range)
N_WIN = R // W_ROWS                 # 4 windows per table
P = 128
EP = 128                            # padded (bf16) row length = 256B
ZF = TOP[0] + 64                    # 1792: zero-padded feature dim
NZCH = ZF // P                      # 14 z chunks
BF16 = ml_dtypes.bfloat16


def _pack_idx_block(idx_i16):
    """[n] int16 (n % 16 == 0) -> [128, n//16]: j -> (j%16, j//16), x8."""
    n = idx_i16.size
    w = idx_i16.reshape(n // 16, 16).T
    return np.tile(w, (8, 1))


def _chunk_weights(wt):
    """W.T [din, dout] -> [128, (din/128)*dout] SBUF chunk layout."""
    din, dout = wt.shape
    nk = din // P
    return np.ascontiguousarray(
        wt.reshape(nk, P, dout).transpose(1, 0, 2).reshape(P, nk * dout))


def _chunk_bias(b):
    """[dout] -> [128, ceil(dout/128)]."""
    dout = b.size
    nch = -(-dout // P)
    buf = np.zeros(nch * P, np.float32)
    buf[:dout] = b
    return np.ascontiguousarray(buf.reshape(nch, P).T)


def _host_prep(x_indices):
    """Segment packing, one gather block per (t, w) spanning 4 bag-tiles.

    Returns caps[t][w][bt] (shared), per-core idx/bagid matrices, tot16,
    gtot.  Global group order: (t, w, bt, g).
    """
    idx = np.asarray(x_indices).astype(np.int64)  # [T, B, L]
    segs = [[[None] * N_CORES for _ in range(N_WIN * N_BT)] for _ in range(T)]
    for c in range(N_CORES):
        idx_c = idx[:, c * B_CORE:(c + 1) * B_CORE, :]        # [T, 512, L]
        win = idx_c // W_ROWS
        for t in range(T):
            for w in range(N_WIN):
                for bt in range(N_BT):
                    sub = idx_c[t, bt * 128:(bt + 1) * 128, :]
                    wsub = win[t, bt * 128:(bt + 1) * 128, :]
                    bags, ls = np.nonzero(wsub == w)
                    li = (sub[bags, ls] - w * W_ROWS).astype(np.int16)
                    segs[t][w * N_BT + bt][c] = (li, bags.astype(np.float32))
    # per-(t,w,bt) capacity: max count over cores, 128-aligned, >= 128
    caps = [[[0] * N_BT for _ in range(N_WIN)] for _ in range(T)]
    for t in range(T):
        for w in range(N_WIN):
            for bt in range(N_BT):
                m = max(segs[t][w * N_BT + bt][c][0].size
                        for c in range(N_CORES))
                caps[t][w][bt] = max(128, -(-m // 128) * 128)
    tot = sum(caps[t][w][bt] for t in range(T) for w in range(N_WIN)
              for bt in range(N_BT))
    tot16 = tot // 16
    gtot = tot // 128
    assert max(sum(caps[t][w]) for t in range(T) for w in range(N_WIN)) <= 8192
    idx_mats, bag_mats = [], []
    for c in range(N_CORES):
        im = np.zeros((P, tot16), np.int16)
        bm = np.full((P, gtot), 255.0, BF16)
        o16 = 0
        og = 0
        for t in range(T):
            for w in range(N_WIN):
                blk = []
                for bt in range(N_BT):
                    cp = caps[t][w][bt]
                    li, bags = segs[t][w * N_BT + bt][c]
                    buf = np.zeros(cp, np.int16)
                    buf[:li.size] = li
                    blk.append(buf)
                    bb = np.full(cp, 255.0, np.float32)
                    bb[:bags.size] = bags
                    bm[:, og:og + cp // 128] = \
                        bb.reshape(cp // 128, P).T.astype(BF16)
                    og += cp // 128
                blk = np.concatenate(blk)
                im[:, o16:o16 + blk.size // 16] = _pack_idx_block(blk)
                o16 += blk.size // 16
        idx_mats.append(im)
        bag_mats.append(bm)
    return caps, idx_mats, bag_mats, tot16, gtot


def _build(caps, tot16, gtot):
    nc = bacc.Bacc("TRN2", target_bir_lowering=False, debug=False,
                   enable_asserts=False, num_devices=N_CORES,
                   num_swdge_queues=4)
    dt = mybir.dt.float32
    bf = mybir.dt.bfloat16
    AF = mybir.ActivationFunctionType

    win_d = [nc.dram_tensor(f"win{t}_{w}", [W_ROWS, EP], bf,
                            kind="ExternalInput").ap()
             for t in range(T) for w in range(N_WIN)]
    idx_d = nc.dram_tensor("idxs", [P, tot16], mybir.dt.int16,
                           kind="ExternalInput").ap()
    bag_d = nc.dram_tensor("bagids", [P, gtot], bf, kind="ExternalInput").ap()
    iota_d = nc.dram_tensor("iota", [P, P], bf, kind="ExternalInput").ap()
    xt_d = nc.dram_tensor("xt", [BOT[0], B_CORE], dt, kind="ExternalInput").ap()
    wdims = [(BOT[0], BOT[1]), (BOT[1], BOT[2]), (BOT[2], BOT[3]),
             (ZF, TOP[1]), (TOP[1], TOP[2]), (TOP[2], TOP[3])]
    w_d = [nc.dram_tensor(f"w{i}", [P, (din // P) * dout],
                          dt if i < 3 else bf,
                          kind="ExternalInput").ap()
           for i, (din, dout) in enumerate(wdims)]
    b_d = [nc.dram_tensor(f"b{i}", [P, -(-dout // P)], dt,
                          kind="ExternalInput").ap()
           for i, (_, dout) in enumerate(wdims)]
    out_d = nc.dram_tensor("y", [1, B_CORE], dt, kind="ExternalOutput").ap()

    gn_tw_max = max(sum(caps[t][w]) // 128 for t in range(T)
                    for w in range(N_WIN))

    with tile.TileContext(nc) as tc:
        with tc.tile_pool(name="const", bufs=1) as cpool, \
             tc.tile_pool(name="zp", bufs=1) as zp, \
             tc.tile_pool(name="stg", bufs=6) as stg, \
             tc.tile_pool(name="gp", bufs=5) as gp, \
             tc.tile_pool(name="sp", bufs=5) as sp, \
             tc.tile_pool(name="act", bufs=1) as actp, \
             tc.tile_pool(name="pps", bufs=2, space="PSUM") as pps, \
             tc.tile_pool(name="yps", bufs=1, space="PSUM") as yps, \
             tc.tile_pool(name="mps", bufs=2, space="PSUM") as mps:

            iota = cpool.tile([P, P], bf)
            nc.sync.dma_start(out=iota[:], in_=iota_d[:])

            bagid = cpool.tile([P, gtot], bf)
            nc.scalar.dma_start(out=bagid[:], in_=bag_d[:])

            zt = zp.tile([P, NZCH * B_CORE], bf)
            nc.vector.memset(zt[:], 0.0)

            def load(name, ap_dram, shape, dtype):
                t_ = cpool.tile(shape, dtype, tag=name)
                nc.scalar.dma_start(out=t_[:], in_=ap_dram)
                return t_

            xt = [load(f"xt{k}", xt_d[k * P:(k + 1) * P, :], [P, B_CORE], dt)
                  for k in range(BOT[0] // P)]
            wts = [load(f"w{i}", w_d[i][:, :], [P, (din // P) * dout],
                        dt if i < 3 else bf)
                   for i, (din, dout) in enumerate(wdims)]
            bts = [load(f"b{i}", b_d[i][:, :], [P, -(-dout // P)], dt)
                   for i, (_, dout) in enumerate(wdims)]

            def mlp_layer(src_aps, li, func, out_tag, out_dt):
                din, dout = wdims[li]
                nk = din // P
                outs = []
                for m in range(-(-dout // P)):
                    mm = min(P, dout - m * P)
                    ps = mps.tile([P, B_CORE], dt, space="PSUM", tag="mlp")
                    for k in range(nk):
                        nc.tensor.matmul(
                            out=ps[:mm, :],
                            lhsT=wts[li][:, k * dout + m * P: k * dout + m * P + mm],
                            rhs=src_aps[k],
                            start=(k == 0), stop=(k == nk - 1))
                    o = actp.tile([P, B_CORE], out_dt, tag=f"{out_tag}{m}")
                    nc.scalar.activation(out=o[:mm, :], in_=ps[:mm, :],
                                         func=func, bias=bts[li][0:mm, m:m + 1])
                    outs.append(o)
                return outs

            # bottom MLP (feature-major h.T tiles [128, 512])
            h1 = mlp_layer([t_[:, :] for t_ in xt], 0, AF.Relu, "h1", dt)
            h2 = mlp_layer([t_[:, :] for t_ in h1], 1, AF.Relu, "h2", dt)
            h3 = mlp_layer([t_[:, :] for t_ in h2], 2, AF.Relu, "h3", dt)
            nc.vector.tensor_copy(out=zt[0:64, 0:B_CORE], in_=h3[0][0:64, :])

            # top-MLP layer 1 accumulates chunk-by-chunk inside the table
            # loop (y1 = w3 @ z); psum tiles live across the whole loop
            y1ps = [yps.tile([P, B_CORE], dt, space="PSUM", tag=f"y1m{m}",
                             name=f"y1ps{m}")
                    for m in range(TOP[1] // P)]

            def y1_accum(chunk):
                zk = zt[:, chunk * B_CORE:(chunk + 1) * B_CORE]
                dout = TOP[1]
                for m in range(TOP[1] // P):
                    nc.tensor.matmul(
                        out=y1ps[m][:, :],
                        lhsT=wts[3][:, chunk * dout + m * P: chunk * dout + (m + 1) * P],
                        rhs=zk,
                        start=(chunk == 0), stop=(chunk == NZCH - 1))

            # embedding gather + selection-matmul pooling
            o16 = 0
            og = 0
            for t in range(T):
                fbase = 64 + 64 * t
                ch = fbase // P
                prow = fbase % P
                pooled = pps.tile([P, N_BT * P], dt, space="PSUM",
                                  tag="pooled")
                n_mm = [sum(caps[t][w][bt] for w in range(N_WIN)) // 128
                        for bt in range(N_BT)]
                mm_i = [0] * N_BT
                for w in range(N_WIN):
                    cp_tw = sum(caps[t][w])
                    c16 = cp_tw // 16
                    gn = cp_tw // 128
                    idx_t = stg.tile([P, c16], mybir.dt.int16, tag="idxt",
                                     padded_shape=[P, gn_tw_max * 8])
                    nc.sync.dma_start(out=idx_t[:], in_=idx_d[:, o16:o16 + c16])
                    dst = gp.tile([P, gn, EP], bf, tag="dst",
                                  padded_shape=[P, gn_tw_max, EP])
                    nc.gpsimd.dma_gather(
                        out_ap=dst[:], in_ap=win_d[t * N_WIN + w][:],
                        idxs_ap=idx_t[:], num_idxs=cp_tw, num_idxs_reg=cp_tw,
                        elem_size=EP, single_packet=False,
                        queue_num=(t * N_WIN + w) % 4)
                    sel = sp.tile([P, gn, P], bf, tag="sel",
                                  padded_shape=[P, gn_tw_max, P])
                    nc.vector.tensor_tensor(
                        out=sel[:],
                        in0=bagid[:, og:og + gn, None].to_broadcast([P, gn, P]),
                        in1=iota[:, None, :].to_broadcast([P, gn, P]),
                        op=mybir.AluOpType.is_equal)
                    g = 0
                    for bt in range(N_BT):
                        for _ in range(caps[t][w][bt] // 128):
                            nc.tensor.matmul(
                                out=pooled[prow:prow + 64,
                                           bt * P:(bt + 1) * P],
                                lhsT=dst[:, g, 0:64], rhs=sel[:, g, :],
                                start=(mm_i[bt] == 0),
                                stop=(mm_i[bt] == n_mm[bt] - 1))
                            mm_i[bt] += 1
                            g += 1
                    o16 += c16
                    og += gn
                nc.vector.tensor_copy(
                    out=zt[prow:prow + 64,
                           ch * B_CORE: ch * B_CORE + N_BT * P],
                    in_=pooled[prow:prow + 64, :])
                if t % 2 == 0:
                    y1_accum(t // 2)
            y1_accum(NZCH - 1)

            # top MLP tail: y1 activation, then layers 2-3
            y1 = []
            for m in range(TOP[1] // P):
                o = actp.tile([P, B_CORE], bf, tag=f"y1a{m}", name=f"y1a{m}")
                nc.scalar.activation(out=o[:, :], in_=y1ps[m][:, :],
                                     func=AF.Relu, bias=bts[3][:, m:m + 1])
                y1.append(o)
            y2 = mlp_layer([t_[:, :] for t_ in y1], 4, AF.Relu, "y2", bf)
            ps = mps.tile([P, B_CORE], dt, space="PSUM", tag="mlp")
            nk = TOP[2] // P
            for k in range(nk):
                nc.tensor.matmul(out=ps[:1, :],
                                 lhsT=wts[5][:, k * TOP[3]: k * TOP[3] + 1],
                                 rhs=y2[k][:, :],
                                 start=(k == 0), stop=(k == nk - 1))
            yo = actp.tile([1, B_CORE], dt, tag="yo")
            nc.scalar.activation(out=yo[:], in_=ps[:1, :], func=AF.Sigmoid,
                                 bias=bts[5][0:1, 0:1])
            nc.sync.dma_start(out=out_d[:], in_=yo[:])

    nc.compile()
    return nc


def _win_tensor(emb_bf16_padded, t, w):
    return np.ascontiguousarray(
        emb_bf16_padded[t, w * W_ROWS:(w + 1) * W_ROWS, :])


def _common_inputs(inputs):
    emb = np.asarray(inputs["emb_tables"], np.float32)
    embp = np.zeros((T, R, EP), BF16)
    embp[:, :, :E] = emb.astype(BF16)
    common = {}
    for t in range(T):
        for w in range(N_WIN):
            common[f"win{t}_{w}"] = _win_tensor(embp, t, w)
    w0 = np.asarray(inputs["top_w0"], np.float32)          # [512, 1728]
    w0p = np.zeros((TOP[1], ZF), np.float32)
    w0p[:, :TOP[0]] = w0
    wlist = [np.asarray(inputs["bot_w0"], np.float32).T,
             np.asarray(inputs["bot_w1"], np.float32).T,
             np.asarray(inputs["bot_w2"], np.float32).T,
             w0p.T,
             np.asarray(inputs["top_w1"], np.float32).T,
             np.asarray(inputs["top_w2"], np.float32).T]
    blist = [np.asarray(inputs["bot_b0"], np.float32),
             np.asarray(inputs["bot_b1"], np.float32),
             np.asarray(inputs["bot_b2"], np.float32),
             np.asarray(inputs["top_b0"], np.float32),
             np.asarray(inputs["top_b1"], np.float32),
             np.asarray(inputs["top_b2"], np.float32)]
    for i in range(6):
        cw = _chunk_weights(wlist[i])
        common[f"w{i}"] = cw if i < 3 else cw.astype(BF16)
        common[f"b{i}"] = _chunk_bias(blist[i])
    common["iota"] = np.broadcast_to(
        np.arange(P, dtype=np.float32), (P, P)).astype(BF16)
    return common


def _prepare(inputs):
    """Returns (nc, in_maps) ready for SPMD execution."""
    x_dense = np.asarray(inputs["x_dense"], np.float32)
    caps, idx_mats, bag_mats, tot16, gtot = _host_prep(inputs["x_indices"])
    nc = _build(caps, tot16, gtot)
    common = _common_inputs(inputs)
    in_maps = []
    for c in range(N_CORES):
        m = dict(common)
        m["idxs"] = idx_mats[c]
        m["bagids"] = bag_mats[c]
        m["xt"] = np.ascontiguousarray(
            x_dense[c * B_CORE:(c + 1) * B_CORE, :].T)
        in_maps.append(m)
    return nc, in_maps


def kernel(**inputs):
    nc, in_maps = _prepare(inputs)
    res = run_bass_kernel_spmd(nc, in_maps, core_ids=list(range(N_CORES)))
    y = np.empty((B, 1), np.float32)
    for c in range(N_CORES):
        y[c * B_CORE:(c + 1) * B_CORE, 0] = res.results[c]["y"][0]
    return y
